# revision 51
# baseline (speedup 1.0000x reference)
"""Trainium2 Bass kernel for nn_BClassifier (spiking MLP classifier).

Data-parallel over batch: 128 samples -> 16 per NeuronCore (8 cores).

Default configuration (KERNEL_MODE=jsplit):
  Pair cores (2c, 2c+1) split the HIDDEN dim: each core computes
  h = x @ W1.T + b1 for BOTH batches of its pair (800 moving columns) over
  its 8 j-tiles (1024 hidden units) with the full F=12288 contraction
  local. All fc1 operands are fp16 (verified bit-exact end-to-end on this
  instance; fp16 matmul runs at the same 1 col/cycle as float32r while W1
  streams at half the bytes), accumulating in fp32 PSUM. The hidden LIF
  scan runs on DVE for the j-half x both batches; spikes are stored fp16 so
  the output matmuls also run at full rate. fc1 is ordered in asymmetric
  j-quarters (4, 3) plus a time-sliced final j-tile (3 passes, W re-
  streamed), so every scan except the last 5-step group hides under fc1
  compute. x-chunk DMAs ride the SWDGE (Pool) queue so their descriptor
  generation never serializes W chunks on the shared HWDGE unit. The only
  cross-core traffic is the output-layer partial o = Wo[:, j-half] @ s1 -
  a [4,400]->[2,400] fp32 ReduceScatter (6.4 KB) per pair, with bo folded
  into the even core's RS input (odd cores get bo2=0) so the sum applies
  the bias exactly once; each core then runs the tiny 25-step memo LIF
  scan and spike-count reduction for its own batch. Older k-split
  (KERNEL_MODE=pair, MM_MODE=f32r) and single-core (KERNEL_MODE=single)
  builds are kept for reference.

Infrastructure note: this walrus build accepts only ONE sync wait per
instruction; _legalize_waits splits Tile's multi-waits onto NoOps.
"""

import os
import sys

import numpy as np

sys.path.insert(0, "/opt/trn_rl_repo")

B, T, C, HH, WW = 128, 25, 3, 64, 64
F = C * HH * WW            # 12288
HID, O = 2048, 2
NCORES = 8
BL = B // NCORES           # 16 samples per core
N = T * BL                 # 400 matmul moving columns
KT = F // 128              # 96 contraction tiles
JT = HID // 128            # 16 hidden tiles
KC = 16                    # k-tiles per W1T DMA chunk
NKC = KT // KC             # 12 chunks per hidden tile
BETA = 0.9
THR = 1.0
MM_MODE = os.environ.get("MM_MODE", "f32r")

_cache = {}


def _legalize_waits(nc, mybir):
    """This walrus build supports only ONE sync wait per instruction (the
    TPB EVENTS struct has a single wait slot and codegen refuses more), while
    Tile freely attaches several. Split excess waits onto standalone NoOps
    placed immediately before the instruction on the same engine queue —
    semantically identical (sequencer blocks on each wait in order)."""
    import bass_rust

    n = 0
    for f in nc.m.functions:
        new_blocks = []
        changed = False
        for bb in f.blocks:
            out = []
            for inst in bb.instructions:
                si = inst.sync_info
                if si and len(si.on_wait) > 1:
                    changed = True
                    waits = list(si.on_wait)
                    for w in waits[:-1]:
                        n += 1
                        out.append(mybir.InstNoOp(
                            name=f"WSPLIT-{n}",
                            engine=inst.engine,
                            ins=[], outs=[],
                            sync_info=mybir.SyncInfo(on_wait=[w], on_update=[]),
                        ))
                    inst.sync_info = mybir.SyncInfo(
                        on_wait=[waits[-1]], on_update=list(si.on_update))
                out.append(inst)
            new_blocks.append(bass_rust.BasicBlock(
                name=bb.name, instructions=out,
                IsPredicated=bb.IsPredicated, IsExit=bb.IsExit,
                IsLoopEntry=bb.IsLoopEntry,
            ))
        if changed:
            f.blocks = new_blocks


def _build():
    import concourse.bass as bass
    import concourse.tile as tile
    from concourse import mybir
    from contextlib import ExitStack

    f32 = mybir.dt.float32
    Alu = mybir.AluOpType
    Act = mybir.ActivationFunctionType

    mm_dt = {"f32": f32, "f32r": mybir.dt.float32r}[MM_MODE]

    nc = bass.Bass("TRN2", target_bir_lowering=False, debug=False)
    xt_d = nc.dram_tensor("xt", [F, N], mm_dt, kind="ExternalInput").ap()
    w1t_d = nc.dram_tensor("w1t", [F, HID], mm_dt, kind="ExternalInput").ap()
    b1_d = nc.dram_tensor("b1c", [128, JT], f32, kind="ExternalInput").ap()
    wot_d = nc.dram_tensor("wot", [128, O * JT], f32, kind="ExternalInput").ap()
    bo_d = nc.dram_tensor("bo2", [O, 1], f32, kind="ExternalInput").ap()
    out_d = nc.dram_tensor("out", [O, BL], f32, kind="ExternalOutput").ap()

    with tile.TileContext(nc) as tc, ExitStack() as ctx:
        const_p = ctx.enter_context(tc.tile_pool(name="const", bufs=1))
        xt_p = ctx.enter_context(tc.tile_pool(name="xt", bufs=1))
        w_p = ctx.enter_context(tc.tile_pool(name="w", bufs=3))
        h_p = ctx.enter_context(tc.tile_pool(name="h", bufs=1))
        ps_p = ctx.enter_context(tc.tile_pool(name="ps", bufs=2, space="PSUM"))
        pso_p = ctx.enter_context(tc.tile_pool(name="pso", bufs=1, space="PSUM"))
        sm_p = ctx.enter_context(tc.tile_pool(name="sm", bufs=1))

        b1_sb = const_p.tile([128, JT], f32)
        nc.sync.dma_start(b1_sb[:, :], b1_d)
        wot_sb = const_p.tile([128, O * JT], f32)
        nc.sync.dma_start(wot_sb[:, :], wot_d)
        bo_sb = const_p.tile([O, 1], f32)
        nc.sync.dma_start(bo_sb[:, :], bo_d)

        # x resident in SBUF: [128, 96*400], col block k holds k-tile k.
        xt_sb = xt_p.tile([128, KT * N], mm_dt)
        xt_r = xt_d.rearrange("(k p) n -> p k n", p=128)  # [128, 96, 400]
        XCH = 12  # k-tiles per chunk DMA
        for ck in range(KT // XCH):
            dst = xt_sb[:, ck * XCH * N:(ck + 1) * XCH * N]
            nc.sync.dma_start(
                dst.rearrange("p (k n) -> p k n", n=N),
                xt_r[:, ck * XCH:(ck + 1) * XCH, :],
            )

        # h (then s1 spikes, in place): [128, 6400], col = t*256 + j*16 + b
        h_all = h_p.tile([128, T * JT * BL], f32)

        # w1t chunk view: [kc, p, s, h]
        w1t_r = w1t_d.rearrange("(kc s p) h -> kc p s h", s=KC, p=128)

        # ---- phase 1: h = x @ W1.T + b1 (transposed: [HID, (t,b)]) ----
        for j in range(JT):
            pt = ps_p.tile([128, N], f32)
            for kc in range(NKC):
                wt = w_p.tile([128, KC * 128], mm_dt)
                nc.sync.dma_start(
                    wt[:, :].rearrange("p (s c) -> p s c", s=KC),
                    w1t_r[kc, :, :, j * 128:(j + 1) * 128],
                )
                for s in range(KC):
                    nc.tensor.matmul(
                        pt[:, :],
                        lhsT=wt[:, s * 128:(s + 1) * 128],
                        rhs=xt_sb[:, (kc * KC + s) * N:(kc * KC + s + 1) * N],
                        start=(kc == 0 and s == 0),
                        stop=(kc == NKC - 1 and s == KC - 1),
                    )
            dst = h_all[:, :].rearrange("p (t g b) -> p t g b", t=T, g=JT)[:, :, j, :]
            nc.scalar.activation(
                dst,
                pt[:, :].rearrange("p (t b) -> p t b", t=T),
                Act.Identity,
                bias=b1_sb[:, j:j + 1],
                scale=1.0,
            )

        # ---- phase 2: hidden LIF scan; spikes overwrite h_all in place ----
        _phases = int(os.environ.get("KERNEL_PHASES", "4"))
        if _phases < 2:
            res = sm_p.tile([O, BL], f32)
            nc.vector.tensor_copy(res[:, :], h_all[0:O, 0:BL])
            nc.sync.dma_start(out_d, res[:, :])
            ctx.close()
            tc.schedule_and_allocate()
            _legalize_waits(nc, mybir)
            return nc
        mem1 = sm_p.tile([128, JT * BL], f32)
        ht = lambda t: h_all[:, t * JT * BL:(t + 1) * JT * BL]
        # t=0: mem1 = h_0 (state starts at 0); s1_0 = (mem1 > 1)
        nc.vector.tensor_copy(mem1[:, :], ht(0))
        nc.vector.tensor_scalar(ht(0), mem1[:, :], THR, None, Alu.is_gt)
        for t in range(1, T):
            # mem1 = beta*mem1 + h_t
            nc.vector.scalar_tensor_tensor(
                mem1[:, :], mem1[:, :], BETA, ht(t), Alu.mult, Alu.add
            )
            # mem1 -= s1_{t-1} (reset by subtraction, THR=1)
            nc.vector.tensor_tensor(mem1[:, :], mem1[:, :], ht(t - 1), Alu.subtract)
            # s1_t = (mem1 > 1), stored over h_t
            nc.vector.tensor_scalar(ht(t), mem1[:, :], THR, None, Alu.is_gt)

        # ---- phase 3: o[(o),(t,b)] = Wo @ s1 + bo, batched over t ----
        po = pso_p.tile([O, N], f32)
        s1_r = h_all[:, :].rearrange("p (t g b) -> p t g b", t=T, g=JT)
        for j in range(JT):
            nc.tensor.matmul(
                po[:, :],
                lhsT=wot_sb[:, O * j:O * (j + 1)],
                rhs=s1_r[:, :, j, :],
                start=(j == 0),
                stop=(j == JT - 1),
            )
        o_sb = sm_p.tile([O, N], f32)
        nc.vector.tensor_scalar(o_sb[:, :], po[:, :], bo_sb[:, 0:1], None, Alu.add)

        # ---- phase 4: output LIF scan on [2, 400], then reduce over t ----
        memo = sm_p.tile([O, BL], f32)
        so_all = sm_p.tile([O, N], f32)
        ot = lambda t: o_sb[:, t * BL:(t + 1) * BL]
        st = lambda t: so_all[:, t * BL:(t + 1) * BL]
        nc.vector.tensor_copy(memo[:, :], ot(0))
        nc.vector.tensor_scalar(st(0), memo[:, :], THR, None, Alu.is_gt)
        for t in range(1, T):
            nc.vector.scalar_tensor_tensor(
                memo[:, :], memo[:, :], BETA, ot(t), Alu.mult, Alu.add
            )
            nc.vector.tensor_tensor(memo[:, :], memo[:, :], st(t - 1), Alu.subtract)
            nc.vector.tensor_scalar(st(t), memo[:, :], THR, None, Alu.is_gt)

        res = sm_p.tile([O, BL], f32)
        nc.vector.tensor_reduce(
            res[:, :],
            so_all[:, :].rearrange("p (t b) -> p b t", t=T),
            axis=mybir.AxisListType.X,
            op=Alu.add,
        )
        nc.sync.dma_start(out_d, res[:, :])

    _legalize_waits(nc, mybir)
    return nc


def _build_pair():
    """K-split pair mode: HBM-stack partner cores (2c, 2c+1) split the F=12288
    contraction dim in half. Each core streams only half of W1T (50 MB instead
    of 100 MB) and computes partial h for BOTH batches of the pair; a per-pair
    ReduceScatter(add) then gives each core the full h for its own batch."""
    import concourse.bass as bass
    import concourse.tile as tile
    from concourse import mybir
    from contextlib import ExitStack

    f32 = mybir.dt.float32
    Alu = mybir.AluOpType
    Act = mybir.ActivationFunctionType

    mm_dt = {"f32": f32, "f32r": mybir.dt.float32r}[MM_MODE]

    KH = KT // 2              # 48 k-tiles per core
    NW = 800                  # both batches' columns
    PKC = 16                  # k-tiles per W chunk DMA
    NPK = KH // PKC           # 3 chunks per hidden tile

    nc = bass.Bass("TRN2", target_bir_lowering=False, debug=False,
                   num_devices=NCORES)
    xt_d = nc.dram_tensor("xt2b", [KH * 128, NW], mm_dt, kind="ExternalInput").ap()
    w1t_d = nc.dram_tensor("w1th", [KH * 128, HID], mm_dt, kind="ExternalInput").ap()
    b1_d = nc.dram_tensor("b1c", [128, JT], f32, kind="ExternalInput").ap()
    wot_d = nc.dram_tensor("wot", [128, O * JT], f32, kind="ExternalInput").ap()
    bo_d = nc.dram_tensor("bo2", [O, 1], f32, kind="ExternalInput").ap()
    out_d = nc.dram_tensor("out", [O, BL], f32, kind="ExternalOutput").ap()

    with tile.TileContext(nc) as tc, ExitStack() as ctx:
        const_p = ctx.enter_context(tc.tile_pool(name="const", bufs=1))
        xt_p = ctx.enter_context(tc.tile_pool(name="xt", bufs=1))
        w_p = ctx.enter_context(tc.tile_pool(name="w", bufs=3))
        h_p = ctx.enter_context(tc.tile_pool(name="h", bufs=1))
        st_p = ctx.enter_context(tc.tile_pool(name="st", bufs=2))
        ps_p = ctx.enter_context(tc.tile_pool(name="ps", bufs=8, space="PSUM"))
        sm_p = ctx.enter_context(tc.tile_pool(name="sm", bufs=1))
        dram_p = ctx.enter_context(tc.tile_pool(name="dram", bufs=1, space="DRAM"))

        b1_sb = const_p.tile([128, JT], f32)
        wot_sb = const_p.tile([128, O * JT], f32)
        bo_sb = const_p.tile([O, 1], f32)

        def load_consts():
            nc.scalar.dma_start(b1_sb[:, :], b1_d)
            nc.scalar.dma_start(wot_sb[:, :], wot_d)
            nc.scalar.dma_start(bo_sb[:, :], bo_d)

        # x for both batches, resident: [128, 48*800]; chunk DMAs are
        # emitted interleaved with the first quarter's W chunks (below) so
        # the first matmuls are not queued behind the whole x load.
        xt_sb = xt_p.tile([128, KH * NW], mm_dt)
        xt_r = xt_d.rearrange("(k p) n -> p k n", p=128)  # [128, 48, 800]
        XT_CUTS = (0, 2, 5, 8, 16, 24, 32, 40, 48)

        def load_xt_chunk(ck):
            k0, k1 = XT_CUTS[ck], XT_CUTS[ck + 1]
            dst = xt_sb[:, k0 * NW:k1 * NW]
            nc.sync.dma_start(
                dst.rearrange("p (k n) -> p k n", n=NW),
                xt_r[:, k0:k1, :],
            )

        # final h (then spikes in place): [128, 6400], col = j*400 + t*16 + b
        h_all = h_p.tile([128, JT * T * BL], f32)

        w1t_r = w1t_d.rearrange("(kc s p) h -> kc p s h", s=PKC, p=128)

        # partial-h bounce buffers: rows [cg*GJ*128 + jj*128 + p], cols (t,b)
        # asymmetric split: the first RS (j 0-11) overlaps quarter 3 compute;
        # the final RS (j 12-15) is small so the pre-scan tail is short.
        GJS = (12, 4)
        in_b = [dram_p.tile([2 * gj * 128, N], f32, name=f"in_b{i}")
                for i, gj in enumerate(GJS)]
        out_b = [dram_p.tile([gj * 128, N], f32, name=f"out_b{i}")
                 for i, gj in enumerate(GJS)]

        XT_EMITTED = [0] * 8
        # ---- phase 1: partial h for both batches over this core's k-half ----
        # Quarter passes: 4 j-tiles x 2 batch-groups = 8 live PSUM banks per
        # pass; k-outer within the pass so x and W stream progressively (no
        # startup stall on the full x load).
        WKC = 3                       # k-tiles per W chunk
        w1t_r4 = w1t_d.rearrange("(kc s p) h -> kc p s h", s=WKC, p=128)
        for q in range(4):
            ps_cg = [ps_p.tile([128, N], f32, name=f"ps_{q}_{i}", tag="pscg")
                     for i in range(8)]  # index jq*2+cg
            for kc in range(KH // WKC):
                wt = w_p.tile([128, WKC * 512], mm_dt)
                # alternate HWDGE issue queues so DMA setup does not
                # serialize on one sequencer
                dma_eng = nc.sync if kc % 2 == 0 else nc.scalar
                dma_eng.dma_start(
                    wt[:, :].rearrange("p (s c) -> p s c", s=WKC),
                    w1t_r4[kc, :, :, q * 512:(q + 1) * 512],
                )
                if q == 0:
                    for ci, at in enumerate((0, 0, 0, 1, 2, 5, 8, 11)):
                        if at == kc and XT_EMITTED[ci] == 0:
                            XT_EMITTED[ci] = 1
                            load_xt_chunk(ci)
                if q == 0 and kc == 1:
                    load_consts()
                for s in range(WKC):
                    k = kc * WKC + s
                    for jq in range(4):
                        for cg in range(2):
                            nc.tensor.matmul(
                                ps_cg[jq * 2 + cg][:, :],
                                lhsT=wt[:, s * 512 + jq * 128:s * 512 + (jq + 1) * 128],
                                rhs=xt_sb[:, k * NW + cg * N:k * NW + (cg + 1) * N],
                                start=(k == 0),
                                stop=(k == KH - 1),
                            )
            for cg in range(2):
                for jh in range(2):
                    stage = st_p.tile([128, 2 * N], f32,
                                      name=f"stage_{q}_{cg}_{jh}", tag="stage")
                    for ji in range(2):
                        jq = jh * 2 + ji
                        j = 4 * q + jq
                        if cg == 0 or q == 3:
                            # PSUM->SBUF with bias on ScalarE
                            nc.scalar.activation(
                                stage[:, ji * N:(ji + 1) * N],
                                ps_cg[jq * 2 + cg][:, :], Act.Identity,
                                bias=b1_sb[:, j:j + 1], scale=1.0,
                            )
                        else:
                            # ... and on VectorE in parallel (idle in phase 1)
                            nc.vector.tensor_scalar(
                                stage[:, ji * N:(ji + 1) * N],
                                ps_cg[jq * 2 + cg][:, :],
                                b1_sb[:, j:j + 1], None, Alu.add,
                            )
                    half = 0 if q < 3 else 1
                    gj = GJS[half]
                    jj0 = (q - (0 if half == 0 else 3)) * 4 + jh * 2
                    dst = in_b[half][cg * gj * 128 + jj0 * 128:
                                     cg * gj * 128 + (jj0 + 2) * 128, :]
                    nc.gpsimd.dma_start(
                        dst.rearrange("(g p) x -> p g x", p=128),
                        stage[:, :].rearrange("p (g x) -> p g x", g=2),
                    )
            if q in (2, 3):
                half = q - 2
                gj = GJS[half]
                j0 = 0 if half == 0 else 12
                if os.environ.get("SKIP_CC", "0") != "1":
                    nc.gpsimd.collective_compute(
                        "ReduceScatter", Alu.add,
                        replica_groups=[[0, 1], [2, 3], [4, 5], [6, 7]],
                        ins=[in_b[half].opt()], outs=[out_b[half].opt()],
                    )
                dst = h_all[:, j0 * N:(j0 + gj) * N]
                nc.gpsimd.dma_start(
                    dst.rearrange("p (g x) -> p g x", g=gj),
                    out_b[half][:, :].rearrange("(g p) x -> p g x", p=128),
                )

        _phases = int(os.environ.get("KERNEL_PHASES", "4"))
        if _phases < 2:
            res = sm_p.tile([O, BL], f32)
            nc.vector.tensor_copy(res[:, :], h_all[0:O, 0:BL])
            nc.sync.dma_start(out_d, res[:, :])
            ctx.close()
            tc.schedule_and_allocate()
            _legalize_waits(nc, mybir)
            return nc

        # ---- phases 2-4 interleaved in groups of TG timesteps ----
        # DVE runs the hidden LIF scan; as soon as a group's spikes exist,
        # PE computes that group's output-layer matmuls (overlapped with the
        # next group's scan on DVE), and the tiny memo scan for group g-1 is
        # interleaved so DVE never head-of-line blocks on PE.
        TG = 5
        NG = T // TG
        mem1 = sm_p.tile([128, JT * BL], f32)
        h4 = h_all[:, :].rearrange("p (g t b) -> p g t b", g=JT, t=T)
        o_sb = sm_p.tile([O, N], f32)
        memo = sm_p.tile([O, BL], f32)
        so_all = sm_p.tile([O, N], f32)
        ot = lambda t: o_sb[:, t * BL:(t + 1) * BL]
        st = lambda t: so_all[:, t * BL:(t + 1) * BL]

        # The LIF scan is elementwise in the hidden dim, so hid j 0-11
        # (delivered by the first ReduceScatter) is scanned while quarter 3
        # is still on the tensor engine; only the j 12-15 scan remains in
        # the tail after the final (small) ReduceScatter.
        def scan_group(g, j0, j1):
            m = mem1[:, j0 * BL:j1 * BL]
            ht = lambda t: h4[:, j0:j1, t, :]
            for t in range(TG * g, TG * (g + 1)):
                if t == 0:
                    nc.vector.tensor_copy(m, ht(0))
                else:
                    nc.vector.scalar_tensor_tensor(
                        m, m, BETA, ht(t), Alu.mult, Alu.add
                    )
                    nc.vector.tensor_tensor(m, m, ht(t - 1), Alu.subtract)
                nc.vector.tensor_scalar(ht(t), m, THR, None, Alu.is_gt)

        def omm_group(g):
            po = ps_p.tile([O, TG * BL], f32, name=f"po_{g}", tag="pscg")
            for j in range(JT):
                nc.tensor.matmul(
                    po[:, :],
                    lhsT=wot_sb[:, O * j:O * (j + 1)],
                    rhs=h_all[:, j * N + g * TG * BL:j * N + (g + 1) * TG * BL],
                    start=(j == 0),
                    stop=(j == JT - 1),
                )
            return po

        def memo_group(g, po):
            nc.vector.tensor_scalar(
                o_sb[:, g * TG * BL:(g + 1) * TG * BL],
                po[:, :], bo_sb[:, 0:1], None, Alu.add)
            for t in range(TG * g, TG * (g + 1)):
                if t == 0:
                    nc.vector.tensor_copy(memo[:, :], ot(0))
                else:
                    nc.vector.scalar_tensor_tensor(
                        memo[:, :], memo[:, :], BETA, ot(t), Alu.mult, Alu.add
                    )
                    nc.vector.tensor_tensor(
                        memo[:, :], memo[:, :], st(t - 1), Alu.subtract)
                nc.vector.tensor_scalar(st(t), memo[:, :], THR, None, Alu.is_gt)

        for g in range(NG):
            scan_group(g, 0, 12)
        pos = {}
        for g in range(NG):
            scan_group(g, 12, 16)
            if g >= 1:
                memo_group(g - 1, pos[g - 1])
            pos[g] = omm_group(g)
        memo_group(NG - 1, pos[NG - 1])

        res = sm_p.tile([O, BL], f32)
        nc.vector.tensor_reduce(
            res[:, :],
            so_all[:, :].rearrange("p (t b) -> p b t", t=T),
            axis=mybir.AxisListType.X,
            op=Alu.add,
        )
        nc.sync.dma_start(out_d, res[:, :])

    _legalize_waits(nc, mybir)
    return nc


def _build_jsplit():
    """J-split pair mode: HBM-stack partner cores (2c, 2c+1) split the HIDDEN
    dim instead of the contraction. Each core computes h = x @ W1.T + b1 for
    BOTH batches of the pair (800 moving cols) over its 8 j-tiles (1024 hid)
    with the FULL F=12288 contraction local, in fp16 (verified bit-exact on
    this instance; fp16 matmul runs at the same 1 col/cycle as f32r but W1
    streams at half the bytes). The hidden LIF scan for the j-half runs on
    DVE for both batches; the only cross-core data is the output-layer
    partial o = Wo[:, jhalf] @ s1 — a [4,400]->[2,400] fp32 ReduceScatter
    (6.4 KB) per pair instead of the 3.3 MB h exchange k-split needs."""
    import concourse.bass as bass
    import concourse.tile as tile
    from concourse import mybir
    from contextlib import ExitStack

    f32 = mybir.dt.float32
    f16 = mybir.dt.float16
    Alu = mybir.AluOpType
    Act = mybir.ActivationFunctionType

    NW = 800                  # both batches' (t, b) columns
    JL = 8                    # local j-tiles (1024 hidden units)
    WKC = 3                   # k-tiles per W chunk DMA
    NKC = KT // WKC           # 32 chunks per quarter

    nc = bass.Bass("TRN2", target_bir_lowering=False, debug=False,
                   num_devices=NCORES)
    xt_d = nc.dram_tensor("xt2b", [F, NW], f16, kind="ExternalInput").ap()
    w1t_d = nc.dram_tensor("w1th", [F, JL * 128], f16, kind="ExternalInput").ap()
    # j7's W repacked with two k-tiles per row (512B descriptors = full DMA
    # bandwidth; the natural 128-col slice would run at half rate)
    w1j7_d = nc.dram_tensor("w1j7", [F // 2, 256], f16, kind="ExternalInput").ap()
    b1_d = nc.dram_tensor("b1c", [128, JL], f32, kind="ExternalInput").ap()
    wot_d = nc.dram_tensor("wotc", [128, 2 * O * JL], f16, kind="ExternalInput").ap()
    bo_d = nc.dram_tensor("bo2", [O, 1], f32, kind="ExternalInput").ap()
    out_d = nc.dram_tensor("out", [O, BL], f32, kind="ExternalOutput").ap()

    with tile.TileContext(nc) as tc, ExitStack() as ctx:
        const_p = ctx.enter_context(tc.tile_pool(name="const", bufs=1))
        xt_p = ctx.enter_context(tc.tile_pool(name="xt", bufs=1))
        w_p = ctx.enter_context(tc.tile_pool(name="w", bufs=4))
        h_p = ctx.enter_context(tc.tile_pool(name="h", bufs=1))
        s1_p = ctx.enter_context(tc.tile_pool(name="s1", bufs=1))
        ps_p = ctx.enter_context(tc.tile_pool(name="ps", bufs=8, space="PSUM"))
        sm_p = ctx.enter_context(tc.tile_pool(name="sm", bufs=1))
        dram_p = ctx.enter_context(tc.tile_pool(name="dram", bufs=1, space="DRAM"))

        b1_sb = const_p.tile([128, JL], f32)
        wot_sb = const_p.tile([128, 2 * O * JL], f16)
        bo_sb = const_p.tile([O, 1], f32)

        def load_consts():
            nc.scalar.dma_start(b1_sb[:, :], b1_d)
            nc.scalar.dma_start(wot_sb[:, :], wot_d)
            nc.scalar.dma_start(bo_sb[:, :], bo_d)

        # x resident in SBUF (fp16): [128, 96*800], col block k = k-tile k.
        # Small 1-2-k-tile chunks (<=1.14us) on the scalar queue so W chunks
        # (sync queue) interleave finely on the shared DMA engines instead of
        # stalling behind multi-us x transfers.
        xt_sb = xt_p.tile([128, KT * NW], f16)
        xt_r = xt_d.rearrange("(k p) n -> p k n", p=128)  # [128, 96, 800]
        XT_CUTS = [0, 1, 2] + list(range(4, KT + 1, 2))

        def load_xt_chunk(ck):
            k0, k1 = XT_CUTS[ck], XT_CUTS[ck + 1]
            dst = xt_sb[:, k0 * NW:k1 * NW]
            # SWDGE (Pool) queue: keeps x descriptor generation off the shared
            # HWDGE unit so W-chunk descgen never queues behind it
            nc.gpsimd.dma_start(
                dst.rearrange("p (k n) -> p k n", n=NW),
                xt_r[:, k0:k1, :],
            )

        # h: [128, 6400] f32, col = jl*800 + cg*400 + t*16 + b
        h_all = h_p.tile([128, JL * NW], f32)
        # spikes in fp16 (exact 0/1) so the output matmul runs at full rate
        s1_sb = s1_p.tile([128, JL * NW], f16)

        w1t_r = w1t_d.rearrange("(kc s p) h -> kc p s h", s=WKC, p=128)

        # ---- phase 1: h = x @ W1.T + b1, asymmetric j-quarters (4, 3, 1) ----
        # The j0-3 scan hides under quarter 1, j4-6 under quarter 2; only the
        # single-j-tile j7 scan (narrow ops) remains in the tail.
        TG = 5
        NG = T // TG
        QUARTERS = ((0, 4), (4, 7))
        w1t_r1 = w1t_d.rearrange("(k p) h -> p k h", p=128)  # [128, 96, 1024]
        # quarter 0 leads with 1-2-k chunks so the first matmul starts early
        W_CUTS0 = [0, 1, 2, 4] + list(range(7, KT, 3)) + [KT]
        # cap every chunk at 1536 fp16 cols (3 KB/partition) so the w pool
        # stays at 3x3KB per partition
        W_CUTSQ = {0: W_CUTS0, 1: list(range(0, KT + 1, 4)),
                   2: list(range(0, KT + 1, 12))}

        for q, (jl0, jl1) in enumerate(QUARTERS):
            njq = jl1 - jl0
            ps_cg = [ps_p.tile([128, N], f32, name=f"ps_{q}_{i}", tag="pscg")
                     for i in range(njq * 2)]  # index jq*2+cg
            cuts = W_CUTSQ[q]
            xi = 0
            for kc in range(len(cuts) - 1):
                k0, k1 = cuts[kc], cuts[kc + 1]
                wt = w_p.tile([128, (k1 - k0) * njq * 128], f16)
                nc.sync.dma_start(
                    wt[:, :].rearrange("p (k c) -> p k c", k=k1 - k0),
                    w1t_r1[:, k0:k1, jl0 * 128:jl1 * 128],
                )
                if q == 0:
                    while xi < len(XT_CUTS) - 1 and XT_CUTS[xi] <= k1 + 10:
                        load_xt_chunk(xi)
                        xi += 1
                    if kc == 2:
                        load_consts()
                for k in range(k0, k1):
                    for jq in range(njq):
                        for cg in range(2):
                            nc.tensor.matmul(
                                ps_cg[jq * 2 + cg][:, :],
                                lhsT=wt[:, ((k - k0) * njq + jq) * 128:
                                        ((k - k0) * njq + jq + 1) * 128],
                                rhs=xt_sb[:, k * NW + cg * N:k * NW + (cg + 1) * N],
                                start=(k == 0),
                                stop=(k == KT - 1),
                            )
            # drains: PSUM -> h_all with bias; split Act/DVE at quarter
            # boundaries so the next quarter's PSUM slots free fast
            for jq in range(njq):
                for cg in range(2):
                    jl = jl0 + jq
                    dst = h_all[:, jl * NW + cg * N:jl * NW + (cg + 1) * N]
                    if jq >= njq - 2:
                        nc.vector.tensor_scalar(
                            dst, ps_cg[jq * 2 + cg][:, :],
                            b1_sb[:, jl:jl + 1], None, Alu.add,
                        )
                    else:
                        nc.scalar.activation(
                            dst, ps_cg[jq * 2 + cg][:, :], Act.Identity,
                            bias=b1_sb[:, jl:jl + 1], scale=1.0,
                        )

        # ---- j-tile 7, TIME-SLICED into 3 passes (t-groups 01 / 23 / 4) ----
        # W for j7 (3.15 MB) is re-streamed per pass; pass p's h drains right
        # away, so the j7 scan of its groups runs while the next pass is on
        # the PE and only ONE scan group remains after fc1 ends.
        J7PASSES = ((0, 2), (2, 4), (4, 5))
        w1j7_r = w1j7_d.rearrange("(k p) c -> p k c", p=128)  # [128, 48, 256]
        for p7, (g0, g1) in enumerate(J7PASSES):
            ncols = (g1 - g0) * TG * BL
            c0 = g0 * TG * BL
            ps_7 = [ps_p.tile([128, ncols], f32, name=f"ps7_{p7}_{cg}",
                              tag="pscg") for cg in range(2)]
            cuts = W_CUTSQ[2]
            for kc in range(len(cuts) - 1):
                k0, k1 = cuts[kc], cuts[kc + 1]
                wt = w_p.tile([128, (k1 - k0) * 128], f16)
                nc.sync.dma_start(
                    wt[:, :].rearrange("p (k c) -> p k c", k=(k1 - k0) // 2),
                    w1j7_r[:, k0 // 2:k1 // 2, :],
                )
                for k in range(k0, k1):
                    for cg in range(2):
                        nc.tensor.matmul(
                            ps_7[cg][:, :],
                            lhsT=wt[:, (k - k0) * 128:(k - k0 + 1) * 128],
                            rhs=xt_sb[:, k * NW + cg * N + c0:
                                      k * NW + cg * N + c0 + ncols],
                            start=(k == 0),
                            stop=(k == KT - 1),
                        )
            for cg in range(2):
                dst = h_all[:, 7 * NW + cg * N + c0:7 * NW + cg * N + c0 + ncols]
                if p7 == len(J7PASSES) - 1:
                    # last pass drains on DVE: the g4 scan follows on the same
                    # engine with no cross-engine semaphore hop
                    nc.vector.tensor_scalar(
                        dst, ps_7[cg][:, :], b1_sb[:, 7:8], None, Alu.add)
                else:
                    nc.scalar.activation(
                        dst, ps_7[cg][:, :], Act.Identity,
                        bias=b1_sb[:, 7:8], scale=1.0,
                    )

        # ---- phase 2: hidden LIF scan (DVE), spikes to s1_sb (fp16) ----
        mem1 = sm_p.tile([128, JL * 2 * BL], f32)   # (jl, cg, b)
        h5 = h_all[:, :].rearrange("p (j c t b) -> p j c t b", j=JL, c=2, t=T)
        s5 = s1_sb[:, :].rearrange("p (j c t b) -> p j c t b", j=JL, c=2, t=T)
        m3 = mem1[:, :].rearrange("p (j c b) -> p j c b", j=JL, c=2)

        def scan_group(g, j0, j1):
            m = m3[:, j0:j1, :, :]
            for t in range(TG * g, TG * (g + 1)):
                ht = h5[:, j0:j1, :, t, :]
                if t == 0:
                    nc.vector.tensor_copy(m, ht)
                else:
                    nc.vector.scalar_tensor_tensor(
                        m, m, BETA, ht, Alu.mult, Alu.add
                    )
                    nc.vector.tensor_tensor(
                        m, m, s5[:, j0:j1, :, t - 1, :], Alu.subtract)
                nc.vector.tensor_scalar(s5[:, j0:j1, :, t, :], m, THR, None,
                                        Alu.is_gt)

        # ---- phase 3: o partials; phase 4: RS + memo scan ----
        stA = sm_p.tile([O, N], f32)   # partial o for batch A (cg0), (t,b)
        stB = sm_p.tile([O, N], f32)
        in_o = dram_p.tile([2 * O, N], f32, name="in_o")
        out_o = dram_p.tile([O, N], f32, name="out_o")

        def omm_group(g):
            po = ps_p.tile([O, 2 * TG * BL], f32, name=f"po_{g}", tag="pscg")
            for jl in range(JL):
                nc.tensor.matmul(
                    po[:, :],
                    lhsT=wot_sb[:, 2 * O * jl:2 * O * jl + O],
                    rhs=s5[:, jl, :, TG * g:TG * (g + 1), :],
                    start=(jl == 0),
                    stop=(jl == JL - 1),
                )
            # bo rides the stage copy; odd cores receive bo2=0 so the
            # ReduceScatter sum applies the bias exactly once per batch
            nc.vector.tensor_scalar(stA[:, g * TG * BL:(g + 1) * TG * BL],
                                    po[:, 0:TG * BL],
                                    bo_sb[:, 0:1], None, Alu.add)
            nc.vector.tensor_scalar(stB[:, g * TG * BL:(g + 1) * TG * BL],
                                    po[:, TG * BL:2 * TG * BL],
                                    bo_sb[:, 0:1], None, Alu.add)

        for g in range(NG):
            scan_group(g, 0, 4)
        for g in range(NG):
            scan_group(g, 4, 7)
        for g in range(NG):
            scan_group(g, 7, 8)
            omm_group(g)

        # one on HWDGE, one on SWDGE so descriptor generation runs in parallel
        nc.sync.dma_start(in_o[0:O, :], stA[:, :])
        nc.gpsimd.dma_start(in_o[O:2 * O, :], stB[:, :])
        if os.environ.get("SKIP_CC", "0") != "1":
            nc.gpsimd.collective_compute(
                "ReduceScatter", Alu.add,
                replica_groups=[[0, 1], [2, 3], [4, 5], [6, 7]],
                ins=[in_o.opt()], outs=[out_o.opt()],
            )
        o_sb = sm_p.tile([O, N], f32)
        nc.sync.dma_start(o_sb[:, :], out_o[:, :])

        memo = sm_p.tile([O, BL], f32)
        so_all = sm_p.tile([O, N], f32)
        ot = lambda t: o_sb[:, t * BL:(t + 1) * BL]
        st = lambda t: so_all[:, t * BL:(t + 1) * BL]
        nc.vector.tensor_copy(memo[:, :], ot(0))
        nc.vector.tensor_scalar(st(0), memo[:, :], THR, None, Alu.is_gt)
        for t in range(1, T):
            nc.vector.scalar_tensor_tensor(
                memo[:, :], memo[:, :], BETA, ot(t), Alu.mult, Alu.add
            )
            nc.vector.tensor_tensor(memo[:, :], memo[:, :], st(t - 1),
                                    Alu.subtract)
            nc.vector.tensor_scalar(st(t), memo[:, :], THR, None, Alu.is_gt)

        res = sm_p.tile([O, BL], f32)
        nc.vector.tensor_reduce(
            res[:, :],
            so_all[:, :].rearrange("p (t b) -> p b t", t=T),
            axis=mybir.AxisListType.X,
            op=Alu.add,
        )
        nc.sync.dma_start(out_d, res[:, :])

    _legalize_waits(nc, mybir)
    return nc


def _prep_inputs_jsplit(x, W1, b1, Wo, bo):
    x = np.ascontiguousarray(x, dtype=np.float32)
    xf = x.reshape(B, T, F)
    w1t = np.ascontiguousarray(W1.T, dtype=np.float16)          # [F, HID]
    in_maps = []
    for c in range(NCORES):
        lo = c & ~1
        half = c & 1
        hid = slice(half * 1024, (half + 1) * 1024)
        xpair = xf[lo * BL:(lo + 2) * BL]                       # [32, 25, F]
        xt2b = np.ascontiguousarray(
            xpair.reshape(2, BL, T, F).transpose(3, 0, 2, 1).reshape(F, 2 * N),
            dtype=np.float16)
        w1th = np.ascontiguousarray(w1t[:, hid])                # [F, 1024] f16
        # j7's 128 cols repacked: [48 kpair x 128 p, 2 parity x 128 c]
        w1j7 = np.ascontiguousarray(
            w1th[:, 896:1024].reshape(48, 2, 128, 128)
            .transpose(0, 2, 1, 3).reshape(F // 2, 256))
        b1c = np.ascontiguousarray(
            b1[hid].astype(np.float32).reshape(8, 128).T)       # [128, 8]
        woth = Wo[:, hid].astype(np.float16).reshape(O, 8, 128)
        # duplicate the two output rows -> 4 out partitions per j-tile
        wotc = np.ascontiguousarray(
            np.concatenate([woth, woth], axis=0)                # [4, 8, 128]
            .transpose(2, 1, 0).reshape(128, 8 * 2 * O))        # [128, 32] f16
        # bias folded into the RS input: only the even core carries bo
        bo2 = np.ascontiguousarray(
            (bo if half == 0 else np.zeros_like(bo))
            .astype(np.float32).reshape(O, 1))
        in_maps.append({"xt2b": xt2b, "w1th": w1th, "w1j7": w1j7,
                        "b1c": b1c, "wotc": wotc, "bo2": bo2})
    return in_maps


def _prep_inputs_pair(x, W1, b1, Wo, bo):
    x = np.ascontiguousarray(x, dtype=np.float32)
    xf = x.reshape(B, T, F)
    w1t = np.ascontiguousarray(W1.T, dtype=np.float32)          # [F, HID]
    b1c = np.ascontiguousarray(b1.astype(np.float32).reshape(JT, 128).T)
    b1z = np.zeros_like(b1c)
    wot = np.ascontiguousarray(
        Wo.astype(np.float32).reshape(O, JT, 128).transpose(2, 1, 0).reshape(128, JT * O)
    )
    bo2 = np.ascontiguousarray(bo.astype(np.float32).reshape(O, 1))
    FH = F // 2
    xts = [np.ascontiguousarray(
        xf[c * BL:(c + 1) * BL].transpose(2, 1, 0).reshape(F, N))
        for c in range(NCORES)]
    in_maps = []
    for c in range(NCORES):
        lo = c & ~1
        half = c & 1
        kr = slice(half * FH, (half + 1) * FH)
        xt2b = np.ascontiguousarray(
            np.concatenate([xts[lo][kr], xts[lo + 1][kr]], axis=1))
        w1th = np.ascontiguousarray(w1t[kr])
        in_maps.append({
            "xt2b": xt2b, "w1th": w1th,
            "b1c": (b1c if half == 0 else b1z),
            "wot": wot, "bo2": bo2,
        })
    return in_maps


def _prep_inputs(x, W1, b1, Wo, bo):
    x = np.ascontiguousarray(x, dtype=np.float32)
    xf = x.reshape(B, T, F)
    w1t = np.ascontiguousarray(W1.T, dtype=np.float32)          # [F, HID]
    b1c = np.ascontiguousarray(
        b1.astype(np.float32).reshape(JT, 128).T)               # [128, JT]
    wot = np.ascontiguousarray(
        Wo.astype(np.float32).reshape(O, JT, 128).transpose(2, 1, 0).reshape(128, JT * O)
    )
    bo2 = np.ascontiguousarray(bo.astype(np.float32).reshape(O, 1))
    in_maps = []
    for c in range(NCORES):
        xc = xf[c * BL:(c + 1) * BL]                            # [16, 25, F]
        xt = np.ascontiguousarray(xc.transpose(2, 1, 0).reshape(F, N))
        in_maps.append({"xt": xt, "w1t": w1t, "b1c": b1c, "wot": wot, "bo2": bo2})
    return in_maps


def kernel(x, W1, b1, Wo, bo):
    from concourse import bass_utils

    mode = os.environ.get("KERNEL_MODE", "jsplit")
    if "nc" not in _cache:
        _cache["nc"] = {
            "jsplit": _build_jsplit,
            "pair": _build_pair,
            "single": _build,
        }[mode]()
    nc = _cache["nc"]

    if mode == "jsplit":
        in_maps = _prep_inputs_jsplit(x, W1, b1, Wo, bo)
    elif mode == "pair":
        in_maps = _prep_inputs_pair(x, W1, b1, Wo, bo)
    else:
        in_maps = _prep_inputs(x, W1, b1, Wo, bo)
    trace = os.environ.get("KERNEL_TRACE", "0") == "1"
    # transient device wedges (NRT_EXEC_UNIT_UNRECOVERABLE) recover on retry
    last_exc = None
    for _attempt in range(3):
        try:
            res = bass_utils.run_bass_kernel_spmd(
                nc, in_maps, core_ids=list(range(NCORES)), trace=trace
            )
            break
        except Exception as e:
            last_exc = e
    else:
        raise last_exc
    if trace and res.exec_time_ns is not None:
        print(f"HW exec time: {res.exec_time_ns} ns")
        _cache["exec_time_ns"] = res.exec_time_ns

    out = np.empty((B, O), dtype=np.float32)
    for c in range(NCORES):
        out[c * BL:(c + 1) * BL, :] = res.results[c]["out"].T
    return out

